# revision 10
# baseline (speedup 1.0000x reference)
"""Binary-weight 3x3 conv (sign(weight) then conv2d, pad=1) on 8 TRN2 cores.

Data-parallel over batch: 32 images -> 4 per core; the small binarized
weight is replicated. Per core: implicit GEMM over the 9 filter taps
accumulated in fp32 PSUM; each output tile is [128 co, 8 rows x 56 cols].

x layout in SBUF (per image, per 128-channel half): W-padded rows with a
stride of 57 and shared zero columns:
  offset 0          : shared zero ("col -1" of virtual row -1)
  virtual row v in [-1, 56] at offset 1 + (v+1)*57, 56 data cols + 1 zero col
  rows -1 and 56 are all zero (H padding)
A tap (kh, kw) for output rows r0..r0+7 reads offset 1 + (r0+kh)*WS + (kw-1)
with free dims [[57, 8], [1, 56]]; every matmul is a uniform [128x128]@[128x448].

MODE:
  "bf16": x loaded fp32 (HWDGE) into staging, DVE cast into padded bf16.
  "f32r": x loaded fp32 (HWDGE) directly into the padded fp32 buffer,
          matmul operands bitcast to float32r (1 col/cycle at N>=256).

Blocks are processed in foursomes ([0..3], [4..6]) sharing each weight
load across the group; weights live in 4 per-(ih, oh) tiles so compute can
start after the two oh=0 slices land. Image 0 is loaded in two row chunks
(0..32, 33..55) to shorten the critical path to the first matmul.
"""

import numpy as np
import ml_dtypes

import concourse.bacc as bacc
import concourse.mybir as mybir
import concourse.tile as tile
from concourse.bass_utils import run_bass_kernel_spmd

MODE = "bf16"  # "bf16" | "f32r"

N_CORES = 8
B = 32
BPC = B // N_CORES  # images per core
C = 256
H = W = 56
HW = H * W  # 3136
WS = 57  # padded row stride
XPAD = 1 + 58 * WS + 1  # 3308 (trailing elem so 8*57 row-view slices stay in bounds)
RB = 8  # output rows per block
NBLK = H // RB  # 7
NF = RB * W  # 448 matmul free size
NTAP = 9
FOURS = ((0,), (1,), (2,), (3,), (4,), (5,), (6,))  # per-block PSUM groups
CHUNKS = ((0, 9), (9, 24), (33, 23))  # image-0 load chunks: (start_row, n_rows)

_CACHE = {}


def _build_module(mode):
    wdt = mybir.dt.bfloat16 if mode == "bf16" else mybir.dt.float32
    nc = bacc.Bacc("TRN2", target_bir_lowering=False, debug=False, num_devices=N_CORES)
    x = nc.declare_dram_parameter("x", [BPC, C, H, W], mybir.dt.float32, isOutput=False)
    wt = nc.declare_dram_parameter("wt", [128, 2 * 2 * NTAP * 128], wdt, isOutput=False)
    out = nc.declare_dram_parameter("out", [BPC, C, H, W], mybir.dt.float32, isOutput=True)

    xf = x.ap().rearrange("b c h w -> b c (h w)")  # [4, 256, 3136]
    of = out.ap().rearrange("b c h w -> b c (h w)")

    with tile.TileContext(nc) as tc:
        with (
            tc.tile_pool(name="xpads", bufs=BPC * 2) as xpool,
            tc.tile_pool(name="xstg", bufs=3) as spool,
            tc.tile_pool(name="wts", bufs=4) as wpool,
            tc.tile_pool(name="osb", bufs=4) as opool,
            tc.tile_pool(name="psum", bufs=8, space="PSUM") as ppool,
        ):
            xpads = {}
            wts = {}

            def init_xpad(n, ih):
                t = xpool.tile([128, XPAD], wdt if mode == "bf16" else mybir.dt.float32,
                               tag="xpad", name=f"xpad_{n}_{ih}")
                xpads[(n, ih)] = t
                nc.vector.memset(t[:, 0 : 1 + WS], 0.0)
                nc.vector.memset(t[:, 1 + 57 * WS : XPAD], 0.0)
                trail = t[:, 2 * WS : 2 * WS + 56 * WS].rearrange(
                    "p (h w) -> p h w", w=WS
                )[:, :, 0:1]
                nc.vector.memset(trail, 0.0)

            def load_chunk(n, ih, r0, nr):
                t = xpads[(n, ih)]
                src = xf[n, ih * 128 : (ih + 1) * 128, r0 * W : (r0 + nr) * W]
                dst = t[:, WS + 1 + r0 * WS : WS + 1 + (r0 + nr) * WS].rearrange(
                    "p (h w) -> p h w", w=WS
                )[:, :, 0:W]
                if mode == "bf16":
                    stg = spool.tile([128, nr * W], mybir.dt.float32, tag="xstg",
                                     name=f"xstg_{n}_{ih}_{r0}")
                    nc.scalar.dma_start(stg[:], src)
                    nc.vector.tensor_copy(dst, stg[:])
                else:
                    nc.scalar.dma_start(dst, src)

            def load_weight(ih, oh):
                wtile = wpool.tile([128, NTAP * 128], wdt, tag="wt",
                                   name=f"wt_{ih}_{oh}")
                wts[(ih, oh)] = wtile
                c0 = (ih * 2 + oh) * NTAP * 128
                nc.sync.dma_start(wtile[:], wt.ap()[:, c0 : c0 + NTAP * 128])

            def mm_ap(ap):
                return ap.bitcast(mybir.dt.float32r) if mode == "f32r" else ap

            # PE warmup: dummy matmuls on scratch data while the first input
            # chunk is in flight, so the HAM clock gate releases (1.2->2.4GHz)
            # before the real matmul stream starts (~3.4us of sustained PE
            # activity required).
            warm_sb = wpool.tile([128, 128], wdt, tag="warm_sb")
            nc.vector.memset(warm_sb[:], 0.0)
            warm_ps = ppool.tile([128, NF], mybir.dt.float32, tag="ps",
                                 name="warm_ps")
            for _ in range(45):
                nc.tensor.matmul(warm_ps[:, 0:128], lhsT=mm_ap(warm_sb[:]),
                                 rhs=mm_ap(warm_sb[:]), start=True, stop=True)

            # critical path first: image-0 chunk 0 of both halves, then the
            # oh=0 weight slices, then the rest
            for ih in range(2):
                init_xpad(0, ih)
            for ih in range(2):
                load_chunk(0, ih, *CHUNKS[0])
            load_weight(0, 0)
            load_weight(1, 0)
            for r0, nr in CHUNKS[1:]:
                for ih in range(2):
                    load_chunk(0, ih, r0, nr)
            load_weight(0, 1)
            load_weight(1, 1)
            for n in range(1, BPC):
                for ih in range(2):
                    init_xpad(n, ih)
                    load_chunk(n, ih, 0, H)

            for n in range(BPC):
                for oh in range(2):
                    for blks in FOURS:
                        pss = [
                            ppool.tile([128, NF], mybir.dt.float32, tag="ps",
                                       name=f"ps_{n}_{oh}_{b}")
                            for b in blks
                        ]
                        k = 0
                        for ih in range(2):
                            # kh=1 first so the start=True matmul covers the
                            # whole psum tile (H-trimmed taps only accumulate)
                            for kh in (1, 0, 2):
                                for kw in range(3):
                                    lhsT = mm_ap(
                                        wts[(ih, oh)][:, (kh * 3 + kw) * 128 :
                                                      (kh * 3 + kw + 1) * 128]
                                    )
                                    for j, blk in enumerate(blks):
                                        # H-edge trim: the tap row that falls
                                        # entirely on zero padding is skipped
                                        # (psum stays contiguous)
                                        rs, nr = blk * RB, RB
                                        if blk == 0 and kh == 0:
                                            rs, nr = 1, RB - 1
                                        elif blk == NBLK - 1 and kh == 2:
                                            nr = RB - 1
                                        off = 1 + (rs + kh) * WS + (kw - 1)
                                        rhs = xpads[(n, ih)][
                                            :, off : off + nr * WS
                                        ].rearrange("p (h w) -> p h w", w=WS)[:, :, 0:W]
                                        p0 = (rs - blk * RB) * W
                                        nc.tensor.matmul(
                                            pss[j][:, p0 : p0 + nr * W],
                                            lhsT=lhsT,
                                            rhs=mm_ap(rhs),
                                            start=(k == 0),
                                            stop=(k == 17),
                                        )
                                    k += 1
                        for j, blk in enumerate(blks):
                            osb = opool.tile([128, NF], mybir.dt.float32, tag="osb")
                            nc.scalar.copy(osb[:], pss[j][:])
                            nc.sync.dma_start(
                                of[n, oh * 128 : (oh + 1) * 128,
                                   blk * RB * W : blk * RB * W + NF],
                                osb[:],
                            )

    nc.compile()
    return nc


def _pack_weights(weight: np.ndarray, mode) -> np.ndarray:
    # lhsT tile for (ih, oh, kh, kw): [ci, co] = sign(w)[oh*128+co, ih*128+ci, kh, kw]
    bw = np.sign(weight.astype(np.float32))
    bw = bw.reshape(2, 128, 2, 128, 3, 3)  # [oh, co, ih, ci, kh, kw]
    bw = bw.transpose(3, 2, 0, 4, 5, 1)  # [ci, ih, oh, kh, kw, co]
    bw = np.ascontiguousarray(bw.reshape(128, 2 * 2 * NTAP * 128))
    return bw.astype(ml_dtypes.bfloat16) if mode == "bf16" else bw


def _get_nc():
    key = ("nc", MODE)
    if key not in _CACHE:
        _CACHE[key] = _build_module(MODE)
    return _CACHE[key]


def _run(x: np.ndarray, weight: np.ndarray, **spmd_kwargs):
    nc = _get_nc()
    wt = _pack_weights(weight, MODE)
    x = np.ascontiguousarray(x.astype(np.float32, copy=False))
    in_maps = [
        {"x": x[i * BPC : (i + 1) * BPC], "wt": wt} for i in range(N_CORES)
    ]
    res = run_bass_kernel_spmd(nc, in_maps, list(range(N_CORES)), **spmd_kwargs)
    out = np.concatenate([r["out"] for r in res.results], axis=0)
    return out, res


def kernel(x: np.ndarray, weight: np.ndarray) -> np.ndarray:
    out, _ = _run(x, weight)
    return out
